# revision 12
# baseline (speedup 1.0000x reference)
"""Trainium2 Bass kernel for the EdgeMask problem.

Computes, for h (B,T,N,d), I_full (B,T,N,N), MLP params W1 (2d,hid) b1 (hid,)
W2 (hid,) b2 (1,):
    li = h @ W1[:d]; lj = h @ W1[d:]
    hid = relu(li[:,:,:,None,:] + lj[:,:,None,:,:] + b1)
    M = sigmoid(hid @ W2 + b2);  I_sparse = I_full * M
Returns (I_sparse, M).

Sharding: data-parallel over B across 8 NeuronCores (B=8), no collectives.

Per-core pipeline (per t slice, N=128, d=128, K=32 hidden):
  - PE: ljT-replicated (one matmul, W1b pre-replicated 4x in cols) and a
    li "stack" S[32*gp+k, g] = li[g+32*gp, k] (+b1 via a rank-1 accumulate
    matmul), both in one PSUM tile.
  - Pointwise hid_g = relu(R + S[:, g]) as 32 fp16 tensor_scalar ops
    ([128,128] each, bias read straight from PSUM), split across
    DVE / ACT / GPSIMD.
  - Reduce over k on PE: per column strip q, two accumulating matmuls with
    zero-padded block-diag W2 stationaries (phase h in {0,1}) consume the
    8 hid buffers; result is a COMPACT [128, 512] PSUM tile whose rows
    32q+4h+m hold logits for i = 32m+8q+4h+c at free chunk c.
  - ACT applies sigmoid(+b2) on the full [128,512] tile (junk rows incl.),
    DVE multiplies with the host-prepermuted I tile, both halves land in
    one [128, 1024] fp16 tile, stored permuted; the host unpermutes/casts.
"""

import functools

import numpy as np

import concourse.bass as bass
import concourse.mybir as mybir
import concourse.tile as tile
from concourse import bacc

F32 = mybir.dt.float32
F16 = mybir.dt.float16

B = 8
T = 32
N = 128
D = 128
K = 32  # hidden
NCORES = 8

AFT = mybir.ActivationFunctionType
ALU = mybir.AluOpType

# pointwise split: group g -> engine. roughly DVE 19 / ACT 6 / Pool 7
PW_DVE = 20
PW_ACT = 5
PW_POOL = 7
HID_BUFS = 2
IO_BUFS = 2
OUT_BUFS = 3
R_BUFS = 4
LILJ_BUFS = 3
RED_BUFS = 2
I_BATCH = 4  # slices per I-load DMA
R_ON_POOL = False


def _pw_engine(g):
    # deterministic interleave so each engine's work spreads over the slice
    seq = (["dve"] * PW_DVE + ["act"] * PW_ACT + ["pool"] * PW_POOL)
    return seq[(g * 7) % K]


def _build(t_slices: int = T):
    nc = bacc.Bacc(
        "TRN2", target_bir_lowering=False, debug=False, num_devices=NCORES
    )

    ht_d = nc.dram_tensor("ht", [D, t_slices * N], F16, kind="ExternalInput")
    ip_d = nc.dram_tensor("ip", [t_slices, N, 2 * N], F16, kind="ExternalInput")
    blob_d = nc.dram_tensor("blob", [D, 416], F16, kind="ExternalInput")
    b2col_d = nc.dram_tensor("b2col", [128, 1], F32, kind="ExternalInput")

    # permuted merged output: [..., 0:512] = M, [..., 512:1024] = I_sparse
    mi_d = nc.dram_tensor("mi", [t_slices, N, 4 * N], F16, kind="ExternalOutput")

    with tile.TileContext(nc) as tc:
        with (
            tc.tile_pool(name="const", bufs=1) as cpool,
            tc.tile_pool(name="rsb", bufs=R_BUFS) as rpool,
            tc.tile_pool(name="hid", bufs=HID_BUFS) as hidpool,
            tc.tile_pool(name="io", bufs=IO_BUFS) as iopool,
            tc.tile_pool(name="outp", bufs=OUT_BUFS) as opool,
            tc.tile_pool(name="psum", bufs=1, space="PSUM") as ppool,
        ):
            # first ht chunk = just slice 0 so the pipeline starts ASAP
            htall_sb = cpool.tile([D, t_slices * N], F16)
            nc.sync.dma_start(htall_sb[:, 0:N], ht_d[:, 0:N])

            blob_sb = cpool.tile([D, 416], F16)
            nc.sync.dma_start(blob_sb[:], blob_d[:])
            w1brep_sb = blob_sb[:, 0:128]
            w1a_sb = blob_sb[:, 128:160]
            wd_sbs = [blob_sb[:, 160 + 32 * p : 192 + 32 * p] for p in range(4)]
            b1col_sb = blob_sb[0:1, 288:416]
            ones_sb = cpool.tile([1, K], F16)
            nc.vector.memset(ones_sb[:], 1)
            b2col_sb = cpool.tile([128, 1], F32)
            nc.sync.dma_start(b2col_sb[:], b2col_d[:])
            n_chunks = min(8, t_slices)
            chunk = t_slices * N // n_chunks
            for ci in range(n_chunks):
                lo = max(ci * chunk, N)
                hi = (ci + 1) * chunk
                if hi > lo:
                    nc.sync.dma_start(htall_sb[:, lo:hi], ht_d[:, lo:hi])

            lilj_tiles = {}
            rs_tiles = {}
            ip_tiles = {}
            red_tiles = {}

            def stage_a(t):
                ht_sb = htall_sb[:, t * N : (t + 1) * N]
                lilj_ps = ppool.tile(
                    [128, N + K], F32, tag="lilj", bufs=LILJ_BUFS, name="lilj"
                )
                nc.tensor.matmul(lilj_ps[:, 0:N], w1brep_sb[:], ht_sb)
                for gp in range(4):
                    nc.tensor.matmul(
                        lilj_ps[32 * gp : 32 * gp + 32, N : N + K],
                        w1a_sb[:],
                        ht_sb[:, 32 * gp : 32 * gp + 32],
                        tile_position=(0, 32 * gp),
                        start=True,
                        stop=False,
                        skip_group_check=True,
                    )
                nc.tensor.matmul(
                    lilj_ps[:, N : N + K],
                    b1col_sb[:],
                    ones_sb[:],
                    start=False,
                    stop=True,
                    skip_group_check=True,
                )
                lilj_tiles[t] = lilj_ps
                r_sb = rpool.tile([128, N], F16, tag="r", name="r")
                nc.scalar.copy(r_sb[:], lilj_ps[:, 0:N])
                s_sb = rpool.tile([128, K], F32, tag="s", name="s")
                nc.vector.tensor_copy(s_sb[:], lilj_ps[:, N : N + K])
                rs_tiles[t] = (r_sb, s_sb)
                if t % I_BATCH == 0:
                    ip_sb = iopool.tile(
                        [128, I_BATCH * 2 * N], F16, tag="ip", name="ip"
                    )
                    nc.sync.dma_start(
                        ip_sb[:],
                        ip_d[t : t + I_BATCH].rearrange("t p f -> p t f"),
                    )
                    ip_tiles[t // I_BATCH] = ip_sb

            def stage_b(t):
                lilj_ps = lilj_tiles.pop(t)
                r_sb, s_sb = rs_tiles.pop(t)
                hbufs = [
                    hidpool.tile([128, 2 * N], F16, tag=f"hb{w}", name=f"hb{w}")
                    for w in range(16)
                ]
                for g in range(K):
                    p, rem = divmod(g, 8)
                    q, c = divmod(rem, 2)
                    dst = hbufs[4 * p + q][:, c * N : (c + 1) * N]
                    s_col = s_sb[:, g : g + 1]
                    eng = _pw_engine(g)
                    if eng == "act":
                        nc.scalar.activation(
                            dst, r_sb[:], AFT.Relu, bias=s_col
                        )
                    elif eng == "pool":
                        nc.gpsimd.tensor_scalar(
                            dst, r_sb[:], s_col, 0.0, ALU.add, ALU.max
                        )
                    else:
                        nc.vector.tensor_scalar(
                            dst, r_sb[:], s_col, 0.0, ALU.add, ALU.max
                        )

                red_ps = ppool.tile(
                    [128, 2 * N], F32, tag="red", bufs=RED_BUFS, name="red"
                )
                for q in range(4):
                    for p in range(4):
                        nc.tensor.matmul(
                            red_ps[32 * q : 32 * q + 32, :],
                            wd_sbs[p][:],
                            hbufs[4 * p + q][:],
                            tile_position=(0, 32 * q),
                            start=(p == 0),
                            stop=(p == 3),
                            skip_group_check=True,
                        )
                red_tiles[t] = red_ps

            def stage_c(t):
                red_ps = red_tiles.pop(t)
                mi_sb = opool.tile([128, 4 * N], F16, tag="mi", name="mi")
                nc.scalar.activation(
                    mi_sb[:, 0 : 2 * N], red_ps[:], AFT.Sigmoid,
                    bias=b2col_sb[:, 0:1],
                )
                ip_sb = ip_tiles[t // I_BATCH]
                nc.vector.tensor_tensor(
                    mi_sb[:, 2 * N : 4 * N],
                    mi_sb[:, 0 : 2 * N],
                    ip_sb[:, (t % I_BATCH) * 2 * N : (t % I_BATCH + 1) * 2 * N],
                    ALU.mult,
                )
                nc.sync.dma_start(mi_d[t, :, :], mi_sb[:])

            SKEW = 2
            for t in range(min(SKEW, t_slices)):
                stage_a(t)
            for t in range(t_slices):
                if t >= 1:
                    stage_c(t - 1)
                stage_b(t)
                if t + SKEW < t_slices:
                    stage_a(t + SKEW)
            stage_c(t_slices - 1)

    nc.compile()
    return nc


def make_aux_inputs(W1, b1, W2, b2):
    W1 = np.asarray(W1, np.float32)
    W1a = W1[:D]
    W1b = W1[D:]
    blob = np.zeros((D, 416), np.float16)
    for gp in range(4):
        blob[:, 32 * gp : 32 * gp + 32] = W1b.astype(np.float16)
    blob[:, 128:160] = W1a.astype(np.float16)
    # 4-phase zero-padded block-diag W2: phase p col (4p+m) has W2 at block m
    for p in range(4):
        for m in range(4):
            blob[32 * m : 32 * m + 32, 160 + 32 * p + 4 * p + m] = np.asarray(
                W2, np.float16
            )
    blob[0, 288:416] = np.tile(np.asarray(b1, np.float32), 4).astype(np.float16)
    b2col = np.full((128, 1), np.asarray(b2, np.float32)[0], np.float32)
    return {
        "blob": blob,
        "b2col": b2col,
    }


def _perm_maps():
    """row r = 32q+4p+m (valid for r%32 < 16), chunk c in {0,1} ->
    i = 32m + 8p + 2q + c."""
    rows = []
    i_of = []
    for q in range(4):
        for p in range(4):
            for m in range(4):
                r = 32 * q + 4 * p + m
                rows.append(r)
                i_of.append([32 * m + 8 * p + 2 * q + c for c in range(2)])
    return np.array(rows), np.array(i_of)


ROWS, I_OF = _perm_maps()


def permute_i(ifull_core):
    """I_full (T, N, N) f32 -> permuted fp16 (T, N, 2N) matching the
    on-device layout; junk rows left zero."""
    out = np.zeros((T, 128, 2 * N), np.float16)
    src = ifull_core.astype(np.float16)
    for ridx, r in enumerate(ROWS):
        for c in range(2):
            out[:, r, c * N : (c + 1) * N] = src[:, I_OF[ridx, c], :]
    return out


def unpermute(mi_core):
    """Permuted (T, N, 4N) fp16 -> (I_sparse, M) each (T, N, N) f32."""
    M = np.empty((T, N, N), np.float32)
    Isp = np.empty((T, N, N), np.float32)
    for ridx, r in enumerate(ROWS):
        for c in range(2):
            i = I_OF[ridx, c]
            M[:, i, :] = mi_core[:, r, c * N : (c + 1) * N].astype(np.float32)
            Isp[:, i, :] = mi_core[:, r, 2 * N + c * N : 2 * N + (c + 1) * N].astype(
                np.float32
            )
    return Isp, M


TRACE = False
LAST_RESULTS = None


@functools.lru_cache(maxsize=1)
def _built_nc():
    return _build(T)


def kernel(**inputs):
    from concourse.bass_utils import run_bass_kernel_spmd

    h = np.asarray(inputs["h"])
    # (B, T, N, D) -> (B, D, T*N) so one DMA per core loads all hT
    ht = np.ascontiguousarray(
        np.transpose(h, (0, 3, 1, 2)).reshape(B, D, -1)
    ).astype(np.float16)
    ifull = np.asarray(inputs["I_full"], np.float32)
    aux = make_aux_inputs(
        inputs["W1"], inputs["b1"], inputs["W2"], inputs["b2"]
    )

    nc = _built_nc()
    in_maps = [
        {"ht": ht[cc], "ip": permute_i(ifull[cc]), **aux} for cc in range(NCORES)
    ]
    res = run_bass_kernel_spmd(
        nc, in_maps, core_ids=list(range(NCORES)), trace=TRACE
    )
    global LAST_RESULTS
    LAST_RESULTS = res
    isp = np.empty((B, T, N, N), np.float32)
    m = np.empty((B, T, N, N), np.float32)
    for cc in range(NCORES):
        i_c, m_c = unpermute(res.results[cc]["mi"])
        isp[cc] = i_c
        m[cc] = m_c
    return isp, m


# revision 13
# speedup vs baseline: 1.0635x; 1.0635x over previous
"""Trainium2 Bass kernel for the EdgeMask problem.

Computes, for h (B,T,N,d), I_full (B,T,N,N), MLP params W1 (2d,hid) b1 (hid,)
W2 (hid,) b2 (1,):
    li = h @ W1[:d]; lj = h @ W1[d:]
    hid = relu(li[:,:,:,None,:] + lj[:,:,None,:,:] + b1)
    M = sigmoid(hid @ W2 + b2);  I_sparse = I_full * M
Returns (I_sparse, M).

Sharding: data-parallel over B across 8 NeuronCores (B=8), no collectives.

Per-core pipeline (per t slice, N=128, d=128, K=32 hidden):
  - PE: ljT-replicated (one matmul, W1b pre-replicated 4x in cols) and a
    li "stack" S[32*gp+k, g] = li[g+32*gp, k] (+b1 via a rank-1 accumulate
    matmul), both in one PSUM tile.
  - Pointwise hid_g = relu(R + S[:, g]) as 32 fp16 tensor_scalar ops
    ([128,128] each, bias read straight from PSUM), split across
    DVE / ACT / GPSIMD.
  - Reduce over k on PE: per column strip q, two accumulating matmuls with
    zero-padded block-diag W2 stationaries (phase h in {0,1}) consume the
    8 hid buffers; result is a COMPACT [128, 512] PSUM tile whose rows
    32q+4h+m hold logits for i = 32m+8q+4h+c at free chunk c.
  - ACT applies sigmoid(+b2) on the full [128,512] tile (junk rows incl.),
    DVE multiplies with the host-prepermuted I tile, both halves land in
    one [128, 1024] fp16 tile, stored permuted; the host unpermutes/casts.
"""

import functools

import numpy as np

import concourse.bass as bass
import concourse.mybir as mybir
import concourse.tile as tile
from concourse import bacc

F32 = mybir.dt.float32
F16 = mybir.dt.float16

B = 8
T = 32
N = 128
D = 128
K = 32  # hidden
NCORES = 8

AFT = mybir.ActivationFunctionType
ALU = mybir.AluOpType

# pointwise split: group g -> engine. roughly DVE 19 / ACT 6 / Pool 7
PW_DVE = 19
PW_ACT = 5
PW_POOL = 8
HID_BUFS = 2
IO_BUFS = 2
OUT_BUFS = 3
R_BUFS = 4
LILJ_BUFS = 3
RED_BUFS = 2
I_BATCH = 4  # slices per I-load DMA
R_ON_POOL = False


def _pw_engine(g):
    # deterministic interleave so each engine's work spreads over the slice
    seq = (["dve"] * PW_DVE + ["act"] * PW_ACT + ["pool"] * PW_POOL)
    return seq[(g * 7) % K]


def _build(t_slices: int = T):
    nc = bacc.Bacc(
        "TRN2", target_bir_lowering=False, debug=False, num_devices=NCORES
    )

    ht_d = nc.dram_tensor("ht", [D, t_slices * N], F16, kind="ExternalInput")
    ip_d = nc.dram_tensor("ip", [t_slices, N, 2 * N], F16, kind="ExternalInput")
    blob_d = nc.dram_tensor("blob", [D, 416], F16, kind="ExternalInput")
    b2col_d = nc.dram_tensor("b2col", [128, 1], F32, kind="ExternalInput")

    # permuted merged output: [..., 0:512] = M, [..., 512:1024] = I_sparse
    mi_d = nc.dram_tensor("mi", [t_slices, N, 4 * N], F16, kind="ExternalOutput")

    with tile.TileContext(nc) as tc:
        with (
            tc.tile_pool(name="const", bufs=1) as cpool,
            tc.tile_pool(name="rsb", bufs=R_BUFS) as rpool,
            tc.tile_pool(name="hid", bufs=HID_BUFS) as hidpool,
            tc.tile_pool(name="io", bufs=IO_BUFS) as iopool,
            tc.tile_pool(name="outp", bufs=OUT_BUFS) as opool,
            tc.tile_pool(name="psum", bufs=1, space="PSUM") as ppool,
        ):
            # first ht chunk = just slice 0 so the pipeline starts ASAP
            htall_sb = cpool.tile([D, t_slices * N], F16)
            nc.sync.dma_start(htall_sb[:, 0:N], ht_d[:, 0:N])

            blob_sb = cpool.tile([D, 416], F16)
            nc.sync.dma_start(blob_sb[:], blob_d[:])
            w1brep_sb = blob_sb[:, 0:128]
            w1a_sb = blob_sb[:, 128:160]
            wd_sbs = [blob_sb[:, 160 + 32 * p : 192 + 32 * p] for p in range(4)]
            b1col_sb = blob_sb[0:1, 288:416]
            ones_sb = cpool.tile([1, K], F16)
            nc.vector.memset(ones_sb[:], 1)
            b2col_sb = cpool.tile([128, 1], F32)
            nc.sync.dma_start(b2col_sb[:], b2col_d[:])
            n_chunks = min(8, t_slices)
            chunk = t_slices * N // n_chunks
            for ci in range(n_chunks):
                lo = max(ci * chunk, N)
                hi = (ci + 1) * chunk
                if hi > lo:
                    nc.sync.dma_start(htall_sb[:, lo:hi], ht_d[:, lo:hi])

            lilj_tiles = {}
            rs_tiles = {}
            ip_tiles = {}
            red_tiles = {}

            def stage_a(t):
                ht_sb = htall_sb[:, t * N : (t + 1) * N]
                lilj_ps = ppool.tile(
                    [128, N + K], F32, tag="lilj", bufs=LILJ_BUFS, name="lilj"
                )
                nc.tensor.matmul(lilj_ps[:, 0:N], w1brep_sb[:], ht_sb)
                for gp in range(4):
                    nc.tensor.matmul(
                        lilj_ps[32 * gp : 32 * gp + 32, N : N + K],
                        w1a_sb[:],
                        ht_sb[:, 32 * gp : 32 * gp + 32],
                        tile_position=(0, 32 * gp),
                        start=True,
                        stop=False,
                        skip_group_check=True,
                    )
                nc.tensor.matmul(
                    lilj_ps[:, N : N + K],
                    b1col_sb[:],
                    ones_sb[:],
                    start=False,
                    stop=True,
                    skip_group_check=True,
                )
                lilj_tiles[t] = lilj_ps
                r_sb = rpool.tile([128, N], F16, tag="r", name="r")
                nc.scalar.copy(r_sb[:], lilj_ps[:, 0:N])
                s_sb = rpool.tile([128, K], F32, tag="s", name="s")
                nc.vector.tensor_copy(s_sb[:], lilj_ps[:, N : N + K])
                rs_tiles[t] = (r_sb, s_sb)
                if t % I_BATCH == 0:
                    ip_sb = iopool.tile(
                        [128, I_BATCH * 2 * N], F16, tag="ip", name="ip"
                    )
                    nc.sync.dma_start(
                        ip_sb[:],
                        ip_d[t : t + I_BATCH].rearrange("t p f -> p t f"),
                    )
                    ip_tiles[t // I_BATCH] = ip_sb

            def stage_b(t):
                lilj_ps = lilj_tiles.pop(t)
                r_sb, s_sb = rs_tiles.pop(t)
                hbufs = [
                    hidpool.tile([128, 2 * N], F16, tag=f"hb{w}", name=f"hb{w}")
                    for w in range(16)
                ]
                for g in range(K):
                    p, rem = divmod(g, 8)
                    q, c = divmod(rem, 2)
                    dst = hbufs[4 * p + q][:, c * N : (c + 1) * N]
                    s_col = s_sb[:, g : g + 1]
                    eng = _pw_engine(g)
                    if eng == "act":
                        nc.scalar.activation(
                            dst, r_sb[:], AFT.Relu, bias=s_col
                        )
                    elif eng == "pool":
                        nc.gpsimd.tensor_scalar(
                            dst, r_sb[:], s_col, 0.0, ALU.add, ALU.max
                        )
                    else:
                        nc.vector.tensor_scalar(
                            dst, r_sb[:], s_col, 0.0, ALU.add, ALU.max
                        )

                red_ps = ppool.tile(
                    [128, 2 * N], F32, tag="red", bufs=RED_BUFS, name="red"
                )
                for q in range(4):
                    for p in range(4):
                        nc.tensor.matmul(
                            red_ps[32 * q : 32 * q + 32, :],
                            wd_sbs[p][:],
                            hbufs[4 * p + q][:],
                            tile_position=(0, 32 * q),
                            start=(p == 0),
                            stop=(p == 3),
                            skip_group_check=True,
                        )
                red_tiles[t] = red_ps

            def stage_c(t):
                red_ps = red_tiles.pop(t)
                mi_sb = opool.tile([128, 4 * N], F16, tag="mi", name="mi")
                nc.scalar.activation(
                    mi_sb[:, 0 : 2 * N], red_ps[:], AFT.Sigmoid,
                    bias=b2col_sb[:, 0:1],
                )
                ip_sb = ip_tiles[t // I_BATCH]
                nc.vector.tensor_tensor(
                    mi_sb[:, 2 * N : 4 * N],
                    mi_sb[:, 0 : 2 * N],
                    ip_sb[:, (t % I_BATCH) * 2 * N : (t % I_BATCH + 1) * 2 * N],
                    ALU.mult,
                )
                nc.sync.dma_start(mi_d[t, :, :], mi_sb[:])

            SKEW = 2
            for t in range(min(SKEW, t_slices)):
                stage_a(t)
            for t in range(t_slices):
                if t >= 1:
                    stage_c(t - 1)
                stage_b(t)
                if t + SKEW < t_slices:
                    stage_a(t + SKEW)
            stage_c(t_slices - 1)

    nc.compile()
    return nc


def make_aux_inputs(W1, b1, W2, b2):
    W1 = np.asarray(W1, np.float32)
    W1a = W1[:D]
    W1b = W1[D:]
    blob = np.zeros((D, 416), np.float16)
    for gp in range(4):
        blob[:, 32 * gp : 32 * gp + 32] = W1b.astype(np.float16)
    blob[:, 128:160] = W1a.astype(np.float16)
    # 4-phase zero-padded block-diag W2: phase p col (4p+m) has W2 at block m
    for p in range(4):
        for m in range(4):
            blob[32 * m : 32 * m + 32, 160 + 32 * p + 4 * p + m] = np.asarray(
                W2, np.float16
            )
    blob[0, 288:416] = np.tile(np.asarray(b1, np.float32), 4).astype(np.float16)
    b2col = np.full((128, 1), np.asarray(b2, np.float32)[0], np.float32)
    return {
        "blob": blob,
        "b2col": b2col,
    }


def _perm_maps():
    """row r = 32q+4p+m (valid for r%32 < 16), chunk c in {0,1} ->
    i = 32m + 8p + 2q + c."""
    rows = []
    i_of = []
    for q in range(4):
        for p in range(4):
            for m in range(4):
                r = 32 * q + 4 * p + m
                rows.append(r)
                i_of.append([32 * m + 8 * p + 2 * q + c for c in range(2)])
    return np.array(rows), np.array(i_of)


ROWS, I_OF = _perm_maps()


def permute_i(ifull_core):
    """I_full (T, N, N) f32 -> permuted fp16 (T, N, 2N) matching the
    on-device layout; junk rows left zero."""
    out = np.zeros((T, 128, 2 * N), np.float16)
    src = ifull_core.astype(np.float16)
    for ridx, r in enumerate(ROWS):
        for c in range(2):
            out[:, r, c * N : (c + 1) * N] = src[:, I_OF[ridx, c], :]
    return out


def unpermute(mi_core):
    """Permuted (T, N, 4N) fp16 -> (I_sparse, M) each (T, N, N) f32."""
    M = np.empty((T, N, N), np.float32)
    Isp = np.empty((T, N, N), np.float32)
    for ridx, r in enumerate(ROWS):
        for c in range(2):
            i = I_OF[ridx, c]
            M[:, i, :] = mi_core[:, r, c * N : (c + 1) * N].astype(np.float32)
            Isp[:, i, :] = mi_core[:, r, 2 * N + c * N : 2 * N + (c + 1) * N].astype(
                np.float32
            )
    return Isp, M


TRACE = False
LAST_RESULTS = None


@functools.lru_cache(maxsize=1)
def _built_nc():
    return _build(T)


def kernel(**inputs):
    from concourse.bass_utils import run_bass_kernel_spmd

    h = np.asarray(inputs["h"])
    # (B, T, N, D) -> (B, D, T*N) so one DMA per core loads all hT
    ht = np.ascontiguousarray(
        np.transpose(h, (0, 3, 1, 2)).reshape(B, D, -1)
    ).astype(np.float16)
    ifull = np.asarray(inputs["I_full"], np.float32)
    aux = make_aux_inputs(
        inputs["W1"], inputs["b1"], inputs["W2"], inputs["b2"]
    )

    nc = _built_nc()
    in_maps = [
        {"ht": ht[cc], "ip": permute_i(ifull[cc]), **aux} for cc in range(NCORES)
    ]
    res = run_bass_kernel_spmd(
        nc, in_maps, core_ids=list(range(NCORES)), trace=TRACE
    )
    global LAST_RESULTS
    LAST_RESULTS = res
    isp = np.empty((B, T, N, N), np.float32)
    m = np.empty((B, T, N, N), np.float32)
    for cc in range(NCORES):
        i_c, m_c = unpermute(res.results[cc]["mi"])
        isp[cc] = i_c
        m[cc] = m_c
    return isp, m


# revision 14
# speedup vs baseline: 1.0741x; 1.0100x over previous
"""Trainium2 Bass kernel for the EdgeMask problem.

Computes, for h (B,T,N,d), I_full (B,T,N,N), MLP params W1 (2d,hid) b1 (hid,)
W2 (hid,) b2 (1,):
    li = h @ W1[:d]; lj = h @ W1[d:]
    hid = relu(li[:,:,:,None,:] + lj[:,:,None,:,:] + b1)
    M = sigmoid(hid @ W2 + b2);  I_sparse = I_full * M
Returns (I_sparse, M).

Sharding: data-parallel over B across 8 NeuronCores (B=8), no collectives.

Per-core pipeline (per t slice, N=128, d=128, K=32 hidden):
  - PE: ljT-replicated (one matmul, W1b pre-replicated 4x in cols) and a
    li "stack" S[32*gp+k, g] = li[g+32*gp, k] (+b1 via a rank-1 accumulate
    matmul), both in one PSUM tile.
  - Pointwise hid_g = relu(R + S[:, g]) as 32 fp16 tensor_scalar ops
    ([128,128] each, bias read straight from PSUM), split across
    DVE / ACT / GPSIMD.
  - Reduce over k on PE: per column strip q, two accumulating matmuls with
    zero-padded block-diag W2 stationaries (phase h in {0,1}) consume the
    8 hid buffers; result is a COMPACT [128, 512] PSUM tile whose rows
    32q+4h+m hold logits for i = 32m+8q+4h+c at free chunk c.
  - ACT applies sigmoid(+b2) on the full [128,512] tile (junk rows incl.),
    DVE multiplies with the host-prepermuted I tile, both halves land in
    one [128, 1024] fp16 tile, stored permuted; the host unpermutes/casts.
"""

import functools

import numpy as np

import concourse.bass as bass
import concourse.mybir as mybir
import concourse.tile as tile
from concourse import bacc

F32 = mybir.dt.float32
F16 = mybir.dt.float16

B = 8
T = 32
N = 128
D = 128
K = 32  # hidden
NCORES = 8

AFT = mybir.ActivationFunctionType
ALU = mybir.AluOpType

# pointwise split: group g -> engine. roughly DVE 19 / ACT 6 / Pool 7
PW_DVE = 19
PW_ACT = 5
PW_POOL = 8
HID_BUFS = 2
IO_BUFS = 2
OUT_BUFS = 3
R_BUFS = 4
LILJ_BUFS = 3
RED_BUFS = 2
I_BATCH = 4  # slices per I-load DMA
R_ON_POOL = False


def _pw_engine(g):
    # deterministic interleave so each engine's work spreads over the slice
    seq = (["dve"] * PW_DVE + ["act"] * PW_ACT + ["pool"] * PW_POOL)
    return seq[(g * 7) % K]


def _build(t_slices: int = T):
    nc = bacc.Bacc(
        "TRN2", target_bir_lowering=False, debug=False, num_devices=NCORES
    )

    ht_d = nc.dram_tensor("ht", [D, t_slices * N], F16, kind="ExternalInput")
    ip_d = nc.dram_tensor("ip", [t_slices, N, 2 * N], F16, kind="ExternalInput")
    blob_d = nc.dram_tensor("blob", [D, 416], F16, kind="ExternalInput")
    b2col_d = nc.dram_tensor("b2col", [128, 1], F32, kind="ExternalInput")

    # permuted merged output: [..., 0:512] = M, [..., 512:1024] = I_sparse
    mi_d = nc.dram_tensor("mi", [t_slices, N, 4 * N], F16, kind="ExternalOutput")

    with tile.TileContext(nc) as tc:
        with (
            tc.tile_pool(name="const", bufs=1) as cpool,
            tc.tile_pool(name="rsb", bufs=R_BUFS) as rpool,
            tc.tile_pool(name="hid", bufs=HID_BUFS) as hidpool,
            tc.tile_pool(name="io", bufs=IO_BUFS) as iopool,
            tc.tile_pool(name="outp", bufs=OUT_BUFS) as opool,
            tc.tile_pool(name="psum", bufs=1, space="PSUM") as ppool,
        ):
            # first ht chunk before everything else so slice 0 starts early
            n_chunks = min(8, t_slices)
            chunk = t_slices * N // n_chunks
            htall_sb = cpool.tile([D, t_slices * N], F16)
            nc.sync.dma_start(htall_sb[:, 0:chunk], ht_d[:, 0:chunk])

            blob_sb = cpool.tile([D, 416], F16)
            nc.sync.dma_start(blob_sb[:], blob_d[:])
            w1brep_sb = blob_sb[:, 0:128]
            w1a_sb = blob_sb[:, 128:160]
            wd_sbs = [blob_sb[:, 160 + 32 * p : 192 + 32 * p] for p in range(4)]
            b1col_sb = blob_sb[0:1, 288:416]
            ones_sb = cpool.tile([1, K], F16)
            nc.vector.memset(ones_sb[:], 1)
            b2col_sb = cpool.tile([128, 1], F32)
            nc.sync.dma_start(b2col_sb[:], b2col_d[:])
            for ci in range(1, n_chunks):
                nc.sync.dma_start(
                    htall_sb[:, ci * chunk : (ci + 1) * chunk],
                    ht_d[:, ci * chunk : (ci + 1) * chunk],
                )

            lilj_tiles = {}
            rs_tiles = {}
            ip_tiles = {}
            red_tiles = {}

            def stage_a(t):
                ht_sb = htall_sb[:, t * N : (t + 1) * N]
                lilj_ps = ppool.tile(
                    [128, N + K], F32, tag="lilj", bufs=LILJ_BUFS, name="lilj"
                )
                nc.tensor.matmul(lilj_ps[:, 0:N], w1brep_sb[:], ht_sb)
                for gp in range(4):
                    nc.tensor.matmul(
                        lilj_ps[32 * gp : 32 * gp + 32, N : N + K],
                        w1a_sb[:],
                        ht_sb[:, 32 * gp : 32 * gp + 32],
                        tile_position=(0, 32 * gp),
                        start=True,
                        stop=False,
                        skip_group_check=True,
                    )
                nc.tensor.matmul(
                    lilj_ps[:, N : N + K],
                    b1col_sb[:],
                    ones_sb[:],
                    start=False,
                    stop=True,
                    skip_group_check=True,
                )
                lilj_tiles[t] = lilj_ps
                r_sb = rpool.tile([128, N], F16, tag="r", name="r")
                nc.scalar.copy(r_sb[:], lilj_ps[:, 0:N])
                s_sb = rpool.tile([128, K], F32, tag="s", name="s")
                nc.vector.tensor_copy(s_sb[:], lilj_ps[:, N : N + K])
                rs_tiles[t] = (r_sb, s_sb)
                if t % I_BATCH == 0:
                    ip_sb = iopool.tile(
                        [128, I_BATCH * 2 * N], F16, tag="ip", name="ip"
                    )
                    nc.sync.dma_start(
                        ip_sb[:],
                        ip_d[t : t + I_BATCH].rearrange("t p f -> p t f"),
                    )
                    ip_tiles[t // I_BATCH] = ip_sb

            def stage_b(t):
                lilj_ps = lilj_tiles.pop(t)
                r_sb, s_sb = rs_tiles.pop(t)
                hbufs = [
                    hidpool.tile([128, 2 * N], F16, tag=f"hb{w}", name=f"hb{w}")
                    for w in range(16)
                ]
                for g in range(K):
                    p, rem = divmod(g, 8)
                    q, c = divmod(rem, 2)
                    dst = hbufs[4 * p + q][:, c * N : (c + 1) * N]
                    s_col = s_sb[:, g : g + 1]
                    eng = _pw_engine(g)
                    if eng == "act":
                        nc.scalar.activation(
                            dst, r_sb[:], AFT.Relu, bias=s_col
                        )
                    elif eng == "pool":
                        nc.gpsimd.tensor_scalar(
                            dst, r_sb[:], s_col, 0.0, ALU.add, ALU.max
                        )
                    else:
                        nc.vector.tensor_scalar(
                            dst, r_sb[:], s_col, 0.0, ALU.add, ALU.max
                        )

                red_ps = ppool.tile(
                    [128, 2 * N], F32, tag="red", bufs=RED_BUFS, name="red"
                )
                for q in range(4):
                    for p in range(4):
                        nc.tensor.matmul(
                            red_ps[32 * q : 32 * q + 32, :],
                            wd_sbs[p][:],
                            hbufs[4 * p + q][:],
                            tile_position=(0, 32 * q),
                            start=(p == 0),
                            stop=(p == 3),
                            skip_group_check=True,
                        )
                red_tiles[t] = red_ps

            def stage_c(t):
                red_ps = red_tiles.pop(t)
                mi_sb = opool.tile([128, 4 * N], F16, tag="mi", name="mi")
                nc.scalar.activation(
                    mi_sb[:, 0 : 2 * N], red_ps[:], AFT.Sigmoid,
                    bias=b2col_sb[:, 0:1],
                )
                ip_sb = ip_tiles[t // I_BATCH]
                nc.vector.tensor_tensor(
                    mi_sb[:, 2 * N : 4 * N],
                    mi_sb[:, 0 : 2 * N],
                    ip_sb[:, (t % I_BATCH) * 2 * N : (t % I_BATCH + 1) * 2 * N],
                    ALU.mult,
                )
                nc.sync.dma_start(mi_d[t, :, :], mi_sb[:])

            SKEW = 2
            for t in range(min(SKEW, t_slices)):
                stage_a(t)
            for t in range(t_slices):
                if t >= 1:
                    stage_c(t - 1)
                stage_b(t)
                if t + SKEW < t_slices:
                    stage_a(t + SKEW)
            stage_c(t_slices - 1)

    nc.compile()
    return nc


def make_aux_inputs(W1, b1, W2, b2):
    W1 = np.asarray(W1, np.float32)
    W1a = W1[:D]
    W1b = W1[D:]
    blob = np.zeros((D, 416), np.float16)
    for gp in range(4):
        blob[:, 32 * gp : 32 * gp + 32] = W1b.astype(np.float16)
    blob[:, 128:160] = W1a.astype(np.float16)
    # 4-phase zero-padded block-diag W2: phase p col (4p+m) has W2 at block m
    for p in range(4):
        for m in range(4):
            blob[32 * m : 32 * m + 32, 160 + 32 * p + 4 * p + m] = np.asarray(
                W2, np.float16
            )
    blob[0, 288:416] = np.tile(np.asarray(b1, np.float32), 4).astype(np.float16)
    b2col = np.full((128, 1), np.asarray(b2, np.float32)[0], np.float32)
    return {
        "blob": blob,
        "b2col": b2col,
    }


def _perm_maps():
    """row r = 32q+4p+m (valid for r%32 < 16), chunk c in {0,1} ->
    i = 32m + 8p + 2q + c."""
    rows = []
    i_of = []
    for q in range(4):
        for p in range(4):
            for m in range(4):
                r = 32 * q + 4 * p + m
                rows.append(r)
                i_of.append([32 * m + 8 * p + 2 * q + c for c in range(2)])
    return np.array(rows), np.array(i_of)


ROWS, I_OF = _perm_maps()


def permute_i(ifull_core):
    """I_full (T, N, N) f32 -> permuted fp16 (T, N, 2N) matching the
    on-device layout; junk rows left zero."""
    out = np.zeros((T, 128, 2 * N), np.float16)
    src = ifull_core.astype(np.float16)
    for ridx, r in enumerate(ROWS):
        for c in range(2):
            out[:, r, c * N : (c + 1) * N] = src[:, I_OF[ridx, c], :]
    return out


def unpermute(mi_core):
    """Permuted (T, N, 4N) fp16 -> (I_sparse, M) each (T, N, N) f32."""
    M = np.empty((T, N, N), np.float32)
    Isp = np.empty((T, N, N), np.float32)
    for ridx, r in enumerate(ROWS):
        for c in range(2):
            i = I_OF[ridx, c]
            M[:, i, :] = mi_core[:, r, c * N : (c + 1) * N].astype(np.float32)
            Isp[:, i, :] = mi_core[:, r, 2 * N + c * N : 2 * N + (c + 1) * N].astype(
                np.float32
            )
    return Isp, M


TRACE = False
LAST_RESULTS = None


@functools.lru_cache(maxsize=1)
def _built_nc():
    return _build(T)


def kernel(**inputs):
    from concourse.bass_utils import run_bass_kernel_spmd

    h = np.asarray(inputs["h"])
    # (B, T, N, D) -> (B, D, T*N) so one DMA per core loads all hT
    ht = np.ascontiguousarray(
        np.transpose(h, (0, 3, 1, 2)).reshape(B, D, -1)
    ).astype(np.float16)
    ifull = np.asarray(inputs["I_full"], np.float32)
    aux = make_aux_inputs(
        inputs["W1"], inputs["b1"], inputs["W2"], inputs["b2"]
    )

    nc = _built_nc()
    in_maps = [
        {"ht": ht[cc], "ip": permute_i(ifull[cc]), **aux} for cc in range(NCORES)
    ]
    res = run_bass_kernel_spmd(
        nc, in_maps, core_ids=list(range(NCORES)), trace=TRACE
    )
    global LAST_RESULTS
    LAST_RESULTS = res
    isp = np.empty((B, T, N, N), np.float32)
    m = np.empty((B, T, N, N), np.float32)
    for cc in range(NCORES):
        i_c, m_c = unpermute(res.results[cc]["mi"])
        isp[cc] = i_c
        m[cc] = m_c
    return isp, m
